# revision 27
# baseline (speedup 1.0000x reference)
"""Trainium2 Bass kernel for nn_Microscope (PSF scatter-add).

Sharding: 8 cores = (b in 0..4) x (h-half in {0,1}).  Each core owns output
rows (b, h_half*128 .. +128) and processes every emitter whose patch rows
intersect its 128-row slab (boundary emitters are duplicated to both
h-halves; each core only writes its own rows, so the output is an exact
partition -- no collectives).

Per core (data-specialized program, compiled at call time):
 - emitters sorted by w, packed 6 per "group" into a [128, 448] bf16 staging
   window (emitter s at partitions 21s..21s+21); 4 group-images per load DMA,
   prefetched ~14 batches ahead.  relu folded into host pack.
 - DVE: batched tensor_reduce per load-batch -> per-partition row sums.
 - PE+DVE: per 32-group batch, indicator matmuls + reciprocal produce the
   per-emitter scale (i_val * 1e6 / sum) broadcast to [128, 1] slots.
 - DVE/ACT (2:1): scl = stg * scale, produced 8-28 groups ahead of the PE.
 - PE: per-emitter row-routing matmuls with M=64 stationary slices of a
   static band-diagonal Zs (band s, shifted identity); rhs = scl columns,
   out = psum ring bank organized [8 w x 64 d] in true output coords
   (h/w/d clipping via slice windows).  Ring of 7 banks over 32 w-tiles;
   one matmul may span several contiguous ring banks (split only at the
   ring wrap), ~1.4 matmuls per emitter.
 - ACT: evacuate finished psum tiles -> SBUF -> DMA to DRAM output.
"""

import threading
from contextlib import ExitStack

import ml_dtypes
import numpy as np

import concourse.bass as bass
import concourse.tile as tile
from concourse import bacc, mybir
from concourse import bass_utils

LAST = None
BF16 = mybir.dt.bfloat16
F32 = mybir.dt.float32
AF = mybir.ActivationFunctionType
BF16NP = ml_dtypes.bfloat16

BS, CH, H, W, D = 4, 1, 256, 256, 64
PH, PW, PD = 21, 21, 21
SCALE_MULT = 10000.0 * 100.0  # folded into i_val
HALF = 128          # h rows per core
G = 6               # emitters per staging group (6*21 = 126 partitions)
GB = 32             # groups per normalization batch
LB = 4              # groups per load DMA
NW = 64             # staging windows
NTILES = 32         # 8-w psum tiles covering w in [0, 256)
NBANKS = 7          # ring size (8th bank for the normalization chain)
PATCH_COLS = PW * PD  # 441
WIN = 448           # staging window width
ZW = 148            # width of each band-diagonal constant


def _host_pack(psf_raw, i_val, b, h, w, d):
    cores = []
    for core in range(8):
        b_t, half = core >> 1, core & 1
        lo = half * HALF
        sel = np.where(
            (b == b_t) & (h - PH // 2 <= lo + HALF - 1) & (h + PH // 2 >= lo)
        )[0]
        order = np.argsort(w[sel], kind="stable")
        idx = sel[order]
        ne = len(idx)
        if ne == 0:
            cores.append(None)
            continue
        npad = (-ne) % G
        if npad:
            idx = np.concatenate([idx, np.repeat(idx[:1], npad)])
        ival = i_val[idx].astype(np.float32) * SCALE_MULT
        if npad:
            ival[ne:] = 0.0
        ntot = len(idx)
        ng = ntot // G
        nb = (ng + GB - 1) // GB
        nb4 = (ng + LB - 1) // LB
        # psf packed partition-major per load-batch: [nb4, 126, LB, 441]
        pf = np.maximum(psf_raw[idx], 0.0).reshape(
            ng, G * PH, PATCH_COLS).astype(BF16NP)
        psf_packed = np.zeros((nb4, 128, LB, PATCH_COLS), BF16NP)
        for g in range(ng):
            psf_packed[g // LB, 0:G * PH, g % LB, :] = pf[g]
        ival_p = np.zeros((nb, G, GB), np.float32)
        iv = ival.reshape(ng, G)  # [group, slot]
        for g in range(ng):
            ival_p[g // GB, :, g % GB] = iv[g]
        he, we, de = h[idx], w[idx], d[idx]
        base = he.astype(np.int64) - PH // 2 - lo          # in [-20, 127]
        # output h window: 0/64 with M=64, or crossing -> W=0, M=128
        winp = np.where(base >= 64, 64, 0).astype(np.int64)
        mm = np.where((base > 43) & (base < 64), 128, 64).astype(np.int64)
        c0 = 64 - base + winp                              # in [1, 84]
        k0 = np.maximum(0, 10 - de).astype(np.int64)
        k1 = np.minimum(PD, 74 - de).astype(np.int64)
        dout0 = de.astype(np.int64) - 10 + k0
        j0 = np.maximum(0, 10 - we).astype(np.int64)
        j1 = np.minimum(PW, 266 - we).astype(np.int64)
        t0 = (we.astype(np.int64) - 10 + j0) // 8
        cores.append(dict(ne=ne, ntot=ntot, ng=ng, nb=nb, nb4=nb4,
                          psf=psf_packed, ival=ival_p,
                          winp=winp, mm=mm, c0=c0, k0=k0, k1=k1,
                          dout0=dout0, j0=j0, j1=j1,
                          w=we.astype(np.int64), t0=t0))
    return cores


def _consts():
    p = np.arange(128)[:, None]
    c = np.arange(ZW)[None, :]
    zs = np.stack([
        ((c - 64 == p - 21 * s) & (p // 21 == s) & (p < 126)).astype(BF16NP)
        for s in range(G)])                                 # [6, 128, ZW]
    ind = ((np.arange(128)[:, None] // 21 == np.arange(G)[None, :])
           & (np.arange(128)[:, None] < 126)).astype(np.float32)   # [128, 6]
    indT = np.ascontiguousarray(ind.T)                      # [6, 128]
    return zs, ind, indT


def _build_program(cd):
    ng, nb, nb4, ntot, ne = cd["ng"], cd["nb"], cd["nb4"], cd["ntot"], cd["ne"]
    nc = bacc.Bacc("TRN2", target_bir_lowering=False, debug=False)
    psf_d = nc.dram_tensor("psf", [nb4, 128, LB, PATCH_COLS], BF16,
                           kind="ExternalInput").ap()
    ival_d = nc.dram_tensor("ival", [nb, G, GB], F32, kind="ExternalInput").ap()
    z_d = nc.dram_tensor("zs", [G, 128, ZW], BF16, kind="ExternalInput").ap()
    ind_d = nc.dram_tensor("ind", [128, G], F32, kind="ExternalInput").ap()
    indT_d = nc.dram_tensor("indT", [G, 128], F32, kind="ExternalInput").ap()
    out_d = nc.dram_tensor("out", [HALF, W, D], F32, kind="ExternalOutput").ap()

    with tile.TileContext(nc) as tc:
        with ExitStack() as ctx:
            const = ctx.enter_context(tc.tile_pool(name="const", bufs=1))
            evp = ctx.enter_context(tc.tile_pool(name="evp", bufs=4))
            psum = ctx.enter_context(tc.tile_pool(name="psum", bufs=1, space="PSUM"))

            ind_t = const.tile([128, G], F32)
            nc.gpsimd.dma_start(ind_t[:], ind_d[:])
            indT_t = const.tile([G, 128], F32)
            nc.gpsimd.dma_start(indT_t[:], indT_d[:])
            z_t = const.tile([128, G * ZW], BF16)
            for s in range(G):
                nc.gpsimd.dma_start(z_t[:, ZW * s:ZW * (s + 1)], z_d[s])

            stg = const.tile([128, NW * WIN], BF16)
            scl = const.tile([128, NW * WIN], BF16)

            ring = psum.tile([128, NBANKS * 512], F32)
            ring_r = ring[:].rearrange("p (w d) -> p w d", d=D)
            ps_norm = psum.tile([128, 512], F32)

            rows_t = [const.tile([128, GB], F32, tag=f"rows{i}", name=f"rows{i}")
                      for i in range(2)]
            scale_t = [const.tile([128, GB], F32, tag=f"scale{i}", name=f"scale{i}")
                       for i in range(2)]
            recip_t = [const.tile([G, GB], F32, tag=f"recip{i}", name=f"recip{i}")
                       for i in range(2)]
            ival_t = [const.tile([G, GB], F32, tag=f"ivalt{i}", name=f"ivalt{i}")
                      for i in range(2)]

            def zero_tile(t):
                if not (0 <= t < NTILES):
                    return
                r = t % NBANKS
                nc.vector.memset(ring[:, 512 * r:512 * (r + 1)], 0.0)

            def evac_tile(t):
                if not (0 <= t < NTILES):
                    return
                ev = evp.tile([128, 512], F32, tag="ev", name="ev")
                r = t % NBANKS
                nc.scalar.activation(ev[:], ring[:, 512 * r:512 * (r + 1)],
                                     AF.Copy)
                q = nc.scalar if t % 2 == 0 else nc.gpsimd
                q.dma_start(out_d[:, 8 * t:8 * t + 8, :], ev[:])

            next_dma = 0       # next load-batch (LB groups) to DMA
            next_red = 0       # next group to row-reduce
            next_a1 = 0        # chain phases per norm batch:
            next_a2 = 0        #   a1: ival DMA + MM1 (PE)   a2: recip+mul (DVE)
            next_b1 = 0        #   b1: MM2 (PE)              b2: copy->scale (DVE)
            next_b2 = 0
            scl_done = set()

            def win(tile_, g):
                return tile_[:, WIN * (g % NW):WIN * (g % NW) + PATCH_COLS]

            def dma_batch():
                nonlocal next_dma
                bi = next_dma
                g0 = bi * LB
                ngrp = min(LB, ng - g0)
                w0 = g0 % NW
                assert w0 + ngrp <= NW
                dst = stg[0:128, WIN * w0:WIN * (w0 + ngrp)].rearrange(
                    "p (g c) -> p g c", c=WIN)[:, :, 0:PATCH_COLS]
                src = psf_d[bi, :, 0:ngrp, :]
                nc.sync.dma_start(dst, src)
                next_dma += 1

            def reduce_group(g):
                # batched: reduces groups g..g+ngrp-1 (one LB batch) in one
                # DVE tensor_reduce [128, ngrp, 441] -> [128, ngrp]
                nonlocal next_red
                assert g == next_red
                assert g % LB == 0
                while next_dma < nb4 and next_dma * LB <= g + 2 * LB:
                    dma_batch()
                rt = rows_t[(g // GB) % 2]
                ngrp = min(LB, ng - g)
                w0 = g % NW
                src = stg[:, WIN * w0:WIN * (w0 + ngrp)].rearrange(
                    "p (g c) -> p g c", c=WIN)[:, :, 0:PATCH_COLS]
                nc.vector.tensor_reduce(rt[:, (g % GB):(g % GB) + ngrp],
                                        src, axis=mybir.AxisListType.X,
                                        op=mybir.AluOpType.add)
                next_red += ngrp

            def chain_a1(k):
                nonlocal next_a1
                if next_a1 > k:
                    return
                assert k == next_a1
                rt, ivt = rows_t[k % 2], ival_t[k % 2]
                while next_red <= min(ng - 1, k * GB + GB - 1):
                    reduce_group(next_red)
                nc.gpsimd.dma_start(ivt[:], ival_d[k])
                nc.tensor.matmul(ps_norm[0:G, (k % 2) * GB:(k % 2) * GB + GB],
                                 ind_t[:], rt[:],
                                 start=True, stop=True, skip_group_check=True)
                next_a1 += 1

            def chain_a2(k):
                nonlocal next_a2
                if next_a2 > k:
                    return
                assert k == next_a2
                chain_a1(k)
                rct, ivt = recip_t[k % 2], ival_t[k % 2]
                nc.vector.reciprocal(rct[:], ps_norm[0:G, (k % 2) * GB:(k % 2) * GB + GB])
                nc.vector.tensor_mul(rct[:], rct[:], ivt[:])
                next_a2 += 1

            def chain_b1(k):
                nonlocal next_b1
                if next_b1 > k:
                    return
                assert k == next_b1
                chain_a2(k)
                rct = recip_t[k % 2]
                nc.tensor.matmul(
                    ps_norm[0:128, 64 + (k % 2) * GB:64 + (k % 2) * GB + GB],
                    indT_t[:], rct[:],
                    start=True, stop=True, skip_group_check=True)
                next_b1 += 1

            def chain_b2(k):
                nonlocal next_b2
                if next_b2 > k:
                    return
                assert k == next_b2
                chain_b1(k)
                sct = scale_t[k % 2]
                nc.vector.tensor_copy(
                    sct[:], ps_norm[0:128, 64 + (k % 2) * GB:64 + (k % 2) * GB + GB])
                next_b2 += 1

            def do_scl(g):
                if g in scl_done:
                    return
                k = g // GB
                sc_ap = scale_t[k % 2][:, (g % GB):(g % GB) + 1]
                if g % 2 == 1:
                    # spread the scale multiplies across ACT too (DVE is the
                    # busier engine: reduces + scales)
                    nc.scalar.activation(win(scl, g), win(stg, g), AF.Copy,
                                         scale=sc_ap)
                else:
                    nc.vector.tensor_scalar(
                        win(scl, g), win(stg, g), sc_ap,
                        None, mybir.AluOpType.mult)
                scl_done.add(g)

            def ensure_scaled(g):
                k = g // GB
                gb = g - k * GB
                # catch-up path (warmup / first batches)
                while next_b2 <= k:
                    chain_b2(next_b2)
                # chain for batch k+1: early, so scale_t[k+1] is ready long
                # before the PE reaches that batch's emitters
                if k + 1 < nb:
                    if gb >= 8:
                        chain_a1(k + 1)
                    if gb >= 10:
                        chain_a2(k + 1)
                    if gb >= 11:
                        chain_b1(k + 1)
                    if gb >= 12:
                        chain_b2(k + 1)
                # DMA prefetch runs well ahead of the reduces so they never
                # wait on loads (window ring is 64 groups deep)
                while next_dma < nb4 and next_dma * LB <= g + 56:
                    dma_batch()
                # reduces ~1.5 batches ahead so their real execution is long
                # done before the chain MMs consume them
                red_target = min(ng - 1, g + 48)
                while next_red <= red_target:
                    reduce_group(next_red)
                do_scl(g)
                for gn in range(g + 1, min(ng, g + 16)):
                    if gn // GB < next_b2:
                        do_scl(gn)
                return win(scl, g)

            def emit_emitter_mms(e, sc):
                s = e % G
                winp, m = int(cd["winp"][e]), int(cd["mm"][e])
                c0 = int(cd["c0"][e])
                k0, k1 = int(cd["k0"][e]), int(cd["k1"][e])
                dout0, we = int(cd["dout0"][e]), int(cd["w"][e])
                j0, j1 = int(cd["j0"][e]), int(cd["j1"][e])
                nd = k1 - k0
                lhsT = z_t[:, ZW * s + c0: ZW * s + c0 + m]
                sc3 = sc.rearrange("p (j k) -> p j k", k=PD)
                j = j0
                while j < j1:
                    wout = we - 10 + j
                    t = wout // 8
                    wl = wout - 8 * t
                    r = t % NBANKS
                    # extend through contiguous ring banks; split only at the
                    # ring wrap (slot NBANKS-1 -> 0)
                    nj = min(j1 - j, 8 * (NBANKS - r) - wl)
                    rhs = sc3[:, j:j + nj, k0:k1]
                    out = ring_r[winp:winp + m, 8 * r + wl:8 * r + wl + nj,
                                 dout0:dout0 + nd]
                    nc.tensor.matmul(out, lhsT, rhs, start=False, stop=False,
                                     skip_group_check=True,
                                     tile_position=(0, winp))
                    j += nj

            # ---- warmup fast path: scales for groups 0..7 from the first
            # two load batches, so PE starts ~20us before the full batch-0
            # chain (32 groups) completes.  Purely additive; the full chain
            # later rewrites scale_t[0] with identical values.
            NWARM = 12
            if ng >= NWARM + 1:
                rq8 = const.tile([G, NWARM], F32)
                nc.gpsimd.dma_start(ival_t[0][:], ival_d[0])
                # stream the first ~15 load batches up front so reduces and
                # the batch-0/1 norm chains never wait on DMA
                while next_dma < min(nb4, 15):
                    dma_batch()
                while next_red < NWARM:
                    reduce_group(next_red)
                nc.tensor.matmul(ps_norm[0:G, 448:448 + NWARM], ind_t[:],
                                 rows_t[0][:, 0:NWARM],
                                 start=True, stop=True, skip_group_check=True)
                nc.vector.reciprocal(rq8[:], ps_norm[0:G, 448:448 + NWARM])
                nc.vector.tensor_mul(rq8[:], rq8[:], ival_t[0][:, 0:NWARM])
                nc.tensor.matmul(ps_norm[0:128, 464:464 + NWARM], indT_t[:],
                                 rq8[:],
                                 start=True, stop=True, skip_group_check=True)
                nc.vector.tensor_copy(scale_t[0][:, 0:NWARM],
                                      ps_norm[0:128, 464:464 + NWARM])
                for gn in range(NWARM - 2):
                    do_scl(gn)

            # ---- main schedule ----
            t0s = cd["t0"]
            step = 0
            for t in range(min(6, NTILES)):
                zero_tile(t)
            for e in range(ntot):
                if e >= ne:
                    continue
                s = int(t0s[e])
                while step < s:
                    evac_tile(step)
                    step += 1
                    zero_tile(step + 5)
                sc = ensure_scaled(e // G)
                emit_emitter_mms(e, sc)
            while step < NTILES:
                evac_tile(step)
                step += 1
                zero_tile(step + 5)

    nc.compile()
    return nc


def kernel(psf_raw, i_val, b, c, h, w, d):
    psf_raw = np.asarray(psf_raw)
    i_val = np.asarray(i_val)
    b = np.asarray(b); h = np.asarray(h); w = np.asarray(w); d = np.asarray(d)
    n = psf_raw.shape[0]
    psf_flat = psf_raw.reshape(n, PH, PW, PD)

    cores = _host_pack(psf_flat, i_val, b, h, w, d)
    zs, ind, indT = _consts()

    ncs = [None] * 8
    errs = []

    def build(i):
        try:
            if cores[i] is not None:
                ncs[i] = _build_program(cores[i])
        except BaseException as exc:
            errs.append((i, exc))
            raise

    threads = [threading.Thread(target=build, args=(i,)) for i in range(8)]
    for t in threads:
        t.start()
    for t in threads:
        t.join()
    if errs:
        raise errs[0][1]

    import jax
    devices = jax.devices()
    results = [None] * 8
    in_maps = [None] * 8

    def run(i):
        if ncs[i] is None:
            results[i] = {"out": np.zeros((HALF, W, D), np.float32)}
            return
        cd = cores[i]
        in_maps[i] = {
            "psf": cd["psf"], "ival": cd["ival"],
            "zs": zs, "ind": ind, "indT": indT,
        }
        try:
            with jax.default_device(devices[i]):
                res = bass_utils.run_bass_kernel_spmd(ncs[i], [in_maps[i]],
                                                      core_ids=[0])
            results[i] = res.results[0]
        except BaseException as exc:
            errs.append((i, exc))
            raise

    rthreads = [threading.Thread(target=run, args=(i,)) for i in range(8)]
    for t in rthreads:
        t.start()
    for t in rthreads:
        t.join()
    if errs:
        raise errs[0][1]

    global LAST
    LAST = {"cores": cores, "ncs": ncs, "in_maps": in_maps}

    out = np.zeros((BS, CH, H, W, D), np.float32)
    for core in range(8):
        b_t, half = core >> 1, core & 1
        out[b_t, 0, half * HALF:(half + 1) * HALF] = results[core]["out"]
    return out



# revision 31
# speedup vs baseline: 1.0176x; 1.0176x over previous
"""Trainium2 Bass kernel for nn_Microscope (PSF scatter-add).

Sharding: 8 cores = (b in 0..4) x (h-half in {0,1}).  Each core owns output
rows (b, h_half*128 .. +128) and processes every emitter whose patch rows
intersect its 128-row slab (boundary emitters are duplicated to both
h-halves; each core only writes its own rows, so the output is an exact
partition -- no collectives).

Per core (data-specialized program, compiled at call time):
 - emitters sorted by w, packed 6 per "group" into a [128, 448] bf16 staging
   window (emitter s at partitions 21s..21s+21); 4 group-images per load DMA,
   prefetched ~14 batches ahead.  relu folded into host pack.
 - DVE: batched tensor_reduce per load-batch -> per-partition row sums.
 - PE+DVE: per 32-group batch, indicator matmuls + reciprocal produce the
   per-emitter scale (i_val * 1e6 / sum) broadcast to [128, 1] slots.
 - DVE/ACT (2:1): scl = stg * scale, produced 8-28 groups ahead of the PE.
 - PE: per-emitter row-routing matmuls with M=64 stationary slices of a
   static band-diagonal Zs (band s, shifted identity); rhs = scl columns,
   out = psum ring bank organized [8 w x 64 d] in true output coords
   (h/w/d clipping via slice windows).  Ring of 7 banks over 32 w-tiles;
   one matmul may span several contiguous ring banks (split only at the
   ring wrap), ~1.4 matmuls per emitter.
 - ACT: evacuate finished psum tiles -> SBUF -> DMA to DRAM output.
"""

import threading
from contextlib import ExitStack

import ml_dtypes
import numpy as np

import concourse.bass as bass
import concourse.tile as tile
from concourse import bacc, mybir
from concourse import bass_utils

LAST = None
BF16 = mybir.dt.bfloat16
F32 = mybir.dt.float32
AF = mybir.ActivationFunctionType
BF16NP = ml_dtypes.bfloat16

BS, CH, H, W, D = 4, 1, 256, 256, 64
PH, PW, PD = 21, 21, 21
SCALE_MULT = 10000.0 * 100.0  # folded into i_val
HALF = 128          # h rows per core
G = 6               # emitters per staging group (6*21 = 126 partitions)
GB = 32             # groups per normalization batch
LB = 4              # groups per load DMA
NW = 88             # staging windows
NTILES = 32         # 8-w psum tiles covering w in [0, 256)
NBANKS = 7          # ring size (8th bank for the normalization chain)
PATCH_COLS = PW * PD  # 441
WIN = 448           # staging window width
ZW = 148            # width of each band-diagonal constant


def _host_pack(psf_raw, i_val, b, h, w, d):
    cores = []
    for core in range(8):
        b_t, half = core >> 1, core & 1
        lo = half * HALF
        sel = np.where(
            (b == b_t) & (h - PH // 2 <= lo + HALF - 1) & (h + PH // 2 >= lo)
        )[0]
        order = np.argsort(w[sel], kind="stable")
        idx = sel[order]
        ne = len(idx)
        if ne == 0:
            cores.append(None)
            continue
        npad = (-ne) % G
        if npad:
            idx = np.concatenate([idx, np.repeat(idx[:1], npad)])
        ival = i_val[idx].astype(np.float32) * SCALE_MULT
        if npad:
            ival[ne:] = 0.0
        ntot = len(idx)
        ng = ntot // G
        nb = (ng + GB - 1) // GB
        nb4 = (ng + LB - 1) // LB
        # psf packed partition-major per load-batch: [nb4, 126, LB, 441]
        pf = np.maximum(psf_raw[idx], 0.0).reshape(
            ng, G * PH, PATCH_COLS).astype(BF16NP)
        psf_packed = np.zeros((nb4, 128, LB, PATCH_COLS), BF16NP)
        for g in range(ng):
            psf_packed[g // LB, 0:G * PH, g % LB, :] = pf[g]
        ival_p = np.zeros((nb, G, GB), np.float32)
        iv = ival.reshape(ng, G)  # [group, slot]
        for g in range(ng):
            ival_p[g // GB, :, g % GB] = iv[g]
        he, we, de = h[idx], w[idx], d[idx]
        base = he.astype(np.int64) - PH // 2 - lo          # in [-20, 127]
        # output h window: 0/64 with M=64, or crossing -> W=0, M=128
        winp = np.where(base >= 64, 64, 0).astype(np.int64)
        mm = np.where((base > 43) & (base < 64), 128, 64).astype(np.int64)
        c0 = 64 - base + winp                              # in [1, 84]
        k0 = np.maximum(0, 10 - de).astype(np.int64)
        k1 = np.minimum(PD, 74 - de).astype(np.int64)
        dout0 = de.astype(np.int64) - 10 + k0
        j0 = np.maximum(0, 10 - we).astype(np.int64)
        j1 = np.minimum(PW, 266 - we).astype(np.int64)
        t0 = (we.astype(np.int64) - 10 + j0) // 8
        cores.append(dict(ne=ne, ntot=ntot, ng=ng, nb=nb, nb4=nb4,
                          psf=psf_packed, ival=ival_p,
                          winp=winp, mm=mm, c0=c0, k0=k0, k1=k1,
                          dout0=dout0, j0=j0, j1=j1,
                          w=we.astype(np.int64), t0=t0))
    return cores


def _consts():
    p = np.arange(128)[:, None]
    c = np.arange(ZW)[None, :]
    zs = np.stack([
        ((c - 64 == p - 21 * s) & (p // 21 == s) & (p < 126)).astype(BF16NP)
        for s in range(G)])                                 # [6, 128, ZW]
    ind = ((np.arange(128)[:, None] // 21 == np.arange(G)[None, :])
           & (np.arange(128)[:, None] < 126)).astype(np.float32)   # [128, 6]
    indT = np.ascontiguousarray(ind.T)                      # [6, 128]
    return zs, ind, indT


def _build_program(cd):
    ng, nb, nb4, ntot, ne = cd["ng"], cd["nb"], cd["nb4"], cd["ntot"], cd["ne"]
    nc = bacc.Bacc("TRN2", target_bir_lowering=False, debug=False)
    psf_d = nc.dram_tensor("psf", [nb4, 128, LB, PATCH_COLS], BF16,
                           kind="ExternalInput").ap()
    ival_d = nc.dram_tensor("ival", [nb, G, GB], F32, kind="ExternalInput").ap()
    z_d = nc.dram_tensor("zs", [G, 128, ZW], BF16, kind="ExternalInput").ap()
    ind_d = nc.dram_tensor("ind", [128, G], F32, kind="ExternalInput").ap()
    indT_d = nc.dram_tensor("indT", [G, 128], F32, kind="ExternalInput").ap()
    out_d = nc.dram_tensor("out", [HALF, W, D], F32, kind="ExternalOutput").ap()

    with tile.TileContext(nc) as tc:
        with ExitStack() as ctx:
            const = ctx.enter_context(tc.tile_pool(name="const", bufs=1))
            evp = ctx.enter_context(tc.tile_pool(name="evp", bufs=4))
            psum = ctx.enter_context(tc.tile_pool(name="psum", bufs=1, space="PSUM"))

            ind_t = const.tile([128, G], F32)
            nc.gpsimd.dma_start(ind_t[:], ind_d[:])
            indT_t = const.tile([G, 128], F32)
            nc.gpsimd.dma_start(indT_t[:], indT_d[:])
            z_t = const.tile([128, G * ZW], BF16)
            for s in range(G):
                nc.gpsimd.dma_start(z_t[:, ZW * s:ZW * (s + 1)], z_d[s])

            stg = const.tile([128, NW * WIN], BF16)
            scl = const.tile([128, NW * WIN], BF16)

            ring = psum.tile([128, NBANKS * 512], F32)
            ring_r = ring[:].rearrange("p (w d) -> p w d", d=D)
            ps_norm = psum.tile([128, 512], F32)

            rows_t = [const.tile([128, GB], F32, tag=f"rows{i}", name=f"rows{i}")
                      for i in range(2)]
            scale_t = [const.tile([128, GB], F32, tag=f"scale{i}", name=f"scale{i}")
                       for i in range(2)]
            recip_t = [const.tile([G, GB], F32, tag=f"recip{i}", name=f"recip{i}")
                       for i in range(2)]
            ival_t = [const.tile([G, GB], F32, tag=f"ivalt{i}", name=f"ivalt{i}")
                      for i in range(2)]

            def zero_tile(t):
                # on ACT: keeps the DVE queue free for reduces + scales
                if not (0 <= t < NTILES):
                    return
                r = t % NBANKS
                nc.scalar.memzero(ring[:, 512 * r:512 * (r + 1)])

            def evac_tile(t):
                if not (0 <= t < NTILES):
                    return
                ev = evp.tile([128, 512], F32, tag="ev", name="ev")
                r = t % NBANKS
                nc.scalar.activation(ev[:], ring[:, 512 * r:512 * (r + 1)],
                                     AF.Copy)
                q = nc.scalar if t % 2 == 0 else nc.gpsimd
                q.dma_start(out_d[:, 8 * t:8 * t + 8, :], ev[:])

            next_dma = 0       # next load-batch (LB groups) to DMA
            next_red = 0       # next group to row-reduce
            next_a1 = 0        # chain phases per norm batch:
            next_a2 = 0        #   a1: ival DMA + MM1 (PE)   a2: recip+mul (DVE)
            next_b1 = 0        #   b1: MM2 (PE)              b2: copy->scale (DVE)
            next_b2 = 0
            scl_done = set()

            def win(tile_, g):
                return tile_[:, WIN * (g % NW):WIN * (g % NW) + PATCH_COLS]

            def dma_batch():
                nonlocal next_dma
                bi = next_dma
                g0 = bi * LB
                ngrp = min(LB, ng - g0)
                w0 = g0 % NW
                assert w0 + ngrp <= NW
                dst = stg[0:128, WIN * w0:WIN * (w0 + ngrp)].rearrange(
                    "p (g c) -> p g c", c=WIN)[:, :, 0:PATCH_COLS]
                src = psf_d[bi, :, 0:ngrp, :]
                nc.sync.dma_start(dst, src)
                next_dma += 1

            def reduce_group(g):
                # batched: reduces groups g..g+ngrp-1 (one LB batch) in one
                # DVE tensor_reduce [128, ngrp, 441] -> [128, ngrp]
                nonlocal next_red
                assert g == next_red
                assert g % LB == 0
                while next_dma < nb4 and next_dma * LB <= g + 2 * LB:
                    dma_batch()
                rt = rows_t[(g // GB) % 2]
                ngrp = min(LB, ng - g)
                w0 = g % NW
                src = stg[:, WIN * w0:WIN * (w0 + ngrp)].rearrange(
                    "p (g c) -> p g c", c=WIN)[:, :, 0:PATCH_COLS]
                nc.vector.tensor_reduce(rt[:, (g % GB):(g % GB) + ngrp],
                                        src, axis=mybir.AxisListType.X,
                                        op=mybir.AluOpType.add)
                next_red += ngrp

            def chain_a1(k):
                nonlocal next_a1
                if next_a1 > k:
                    return
                assert k == next_a1
                rt, ivt = rows_t[k % 2], ival_t[k % 2]
                while next_red <= min(ng - 1, k * GB + GB - 1):
                    reduce_group(next_red)
                nc.gpsimd.dma_start(ivt[:], ival_d[k])
                nc.tensor.matmul(ps_norm[0:G, (k % 2) * GB:(k % 2) * GB + GB],
                                 ind_t[:], rt[:],
                                 start=True, stop=True, skip_group_check=True)
                next_a1 += 1

            def chain_a2(k):
                nonlocal next_a2
                if next_a2 > k:
                    return
                assert k == next_a2
                chain_a1(k)
                rct, ivt = recip_t[k % 2], ival_t[k % 2]
                nc.vector.reciprocal(rct[:], ps_norm[0:G, (k % 2) * GB:(k % 2) * GB + GB])
                nc.vector.tensor_mul(rct[:], rct[:], ivt[:])
                next_a2 += 1

            def chain_b1(k):
                nonlocal next_b1
                if next_b1 > k:
                    return
                assert k == next_b1
                chain_a2(k)
                rct = recip_t[k % 2]
                nc.tensor.matmul(
                    ps_norm[0:128, 64 + (k % 2) * GB:64 + (k % 2) * GB + GB],
                    indT_t[:], rct[:],
                    start=True, stop=True, skip_group_check=True)
                next_b1 += 1

            def chain_b2(k):
                nonlocal next_b2
                if next_b2 > k:
                    return
                assert k == next_b2
                chain_b1(k)
                sct = scale_t[k % 2]
                nc.vector.tensor_copy(
                    sct[:], ps_norm[0:128, 64 + (k % 2) * GB:64 + (k % 2) * GB + GB])
                next_b2 += 1

            def do_scl(g):
                if g in scl_done:
                    return
                k = g // GB
                sc_ap = scale_t[k % 2][:, (g % GB):(g % GB) + 1]
                if g % 3 == 2:
                    # spread the scale multiplies across ACT too (DVE is the
                    # busier engine: reduces + scales)
                    nc.scalar.activation(win(scl, g), win(stg, g), AF.Copy,
                                         scale=sc_ap)
                else:
                    nc.vector.tensor_scalar(
                        win(scl, g), win(stg, g), sc_ap,
                        None, mybir.AluOpType.mult)
                scl_done.add(g)

            def ensure_scaled(g):
                k = g // GB
                gb = g - k * GB
                # catch-up path (warmup / first batches)
                while next_b2 <= k:
                    chain_b2(next_b2)
                # chain for batch k+1: early, so scale_t[k+1] is ready long
                # before the PE reaches that batch's emitters
                if k + 1 < nb:
                    if gb >= 8:
                        chain_a1(k + 1)
                    if gb >= 10:
                        chain_a2(k + 1)
                    if gb >= 11:
                        chain_b1(k + 1)
                    if gb >= 12:
                        chain_b2(k + 1)
                # DMA prefetch runs well ahead of the reduces so they never
                # wait on loads (window ring is 64 groups deep)
                while next_dma < nb4 and next_dma * LB <= g + 76:
                    dma_batch()
                # reduces ~1.5 batches ahead so their real execution is long
                # done before the chain MMs consume them
                red_target = min(ng - 1, g + 48)
                while next_red <= red_target:
                    reduce_group(next_red)
                do_scl(g)
                for gn in range(g + 1, min(ng, g + 16)):
                    if gn // GB < next_b2:
                        do_scl(gn)
                return win(scl, g)

            def emit_emitter_mms(e, sc):
                s = e % G
                winp, m = int(cd["winp"][e]), int(cd["mm"][e])
                c0 = int(cd["c0"][e])
                k0, k1 = int(cd["k0"][e]), int(cd["k1"][e])
                dout0, we = int(cd["dout0"][e]), int(cd["w"][e])
                j0, j1 = int(cd["j0"][e]), int(cd["j1"][e])
                nd = k1 - k0
                lhsT = z_t[:, ZW * s + c0: ZW * s + c0 + m]
                sc3 = sc.rearrange("p (j k) -> p j k", k=PD)
                j = j0
                while j < j1:
                    wout = we - 10 + j
                    t = wout // 8
                    wl = wout - 8 * t
                    r = t % NBANKS
                    # extend through contiguous ring banks; split only at the
                    # ring wrap (slot NBANKS-1 -> 0)
                    nj = min(j1 - j, 8 * (NBANKS - r) - wl)
                    rhs = sc3[:, j:j + nj, k0:k1]
                    out = ring_r[winp:winp + m, 8 * r + wl:8 * r + wl + nj,
                                 dout0:dout0 + nd]
                    nc.tensor.matmul(out, lhsT, rhs, start=False, stop=False,
                                     skip_group_check=True,
                                     tile_position=(0, winp))
                    j += nj

            # ---- warmup fast path: scales for groups 0..7 from the first
            # two load batches, so PE starts ~20us before the full batch-0
            # chain (32 groups) completes.  Purely additive; the full chain
            # later rewrites scale_t[0] with identical values.
            NWARM = 12
            if ng >= NWARM + 1:
                rq8 = const.tile([G, NWARM], F32)
                nc.gpsimd.dma_start(ival_t[0][:], ival_d[0])
                # stream the first ~15 load batches up front so reduces and
                # the batch-0/1 norm chains never wait on DMA
                while next_dma < min(nb4, 15):
                    dma_batch()
                while next_red < NWARM:
                    reduce_group(next_red)
                nc.tensor.matmul(ps_norm[0:G, 448:448 + NWARM], ind_t[:],
                                 rows_t[0][:, 0:NWARM],
                                 start=True, stop=True, skip_group_check=True)
                nc.vector.reciprocal(rq8[:], ps_norm[0:G, 448:448 + NWARM])
                nc.vector.tensor_mul(rq8[:], rq8[:], ival_t[0][:, 0:NWARM])
                nc.tensor.matmul(ps_norm[0:128, 464:464 + NWARM], indT_t[:],
                                 rq8[:],
                                 start=True, stop=True, skip_group_check=True)
                nc.vector.tensor_copy(scale_t[0][:, 0:NWARM],
                                      ps_norm[0:128, 464:464 + NWARM])
                for gn in range(NWARM - 2):
                    do_scl(gn)

            # ---- main schedule ----
            t0s = cd["t0"]
            step = 0
            for t in range(min(6, NTILES)):
                zero_tile(t)
            for e in range(ntot):
                if e >= ne:
                    continue
                s = int(t0s[e])
                while step < s:
                    evac_tile(step)
                    step += 1
                    zero_tile(step + 5)
                sc = ensure_scaled(e // G)
                emit_emitter_mms(e, sc)
            while step < NTILES:
                evac_tile(step)
                step += 1
                zero_tile(step + 5)

    nc.compile()
    return nc


def kernel(psf_raw, i_val, b, c, h, w, d):
    psf_raw = np.asarray(psf_raw)
    i_val = np.asarray(i_val)
    b = np.asarray(b); h = np.asarray(h); w = np.asarray(w); d = np.asarray(d)
    n = psf_raw.shape[0]
    psf_flat = psf_raw.reshape(n, PH, PW, PD)

    cores = _host_pack(psf_flat, i_val, b, h, w, d)
    zs, ind, indT = _consts()

    ncs = [None] * 8
    errs = []

    def build(i):
        try:
            if cores[i] is not None:
                ncs[i] = _build_program(cores[i])
        except BaseException as exc:
            errs.append((i, exc))
            raise

    threads = [threading.Thread(target=build, args=(i,)) for i in range(8)]
    for t in threads:
        t.start()
    for t in threads:
        t.join()
    if errs:
        raise errs[0][1]

    import jax
    devices = jax.devices()
    results = [None] * 8
    in_maps = [None] * 8

    def run(i):
        if ncs[i] is None:
            results[i] = {"out": np.zeros((HALF, W, D), np.float32)}
            return
        cd = cores[i]
        in_maps[i] = {
            "psf": cd["psf"], "ival": cd["ival"],
            "zs": zs, "ind": ind, "indT": indT,
        }
        try:
            with jax.default_device(devices[i]):
                res = bass_utils.run_bass_kernel_spmd(ncs[i], [in_maps[i]],
                                                      core_ids=[0])
            results[i] = res.results[0]
        except BaseException as exc:
            errs.append((i, exc))
            raise

    rthreads = [threading.Thread(target=run, args=(i,)) for i in range(8)]
    for t in rthreads:
        t.start()
    for t in rthreads:
        t.join()
    if errs:
        raise errs[0][1]

    global LAST
    LAST = {"cores": cores, "ncs": ncs, "in_maps": in_maps}

    out = np.zeros((BS, CH, H, W, D), np.float32)
    for core in range(8):
        b_t, half = core >> 1, core & 1
        out[b_t, 0, half * HALF:(half + 1) * HALF] = results[core]["out"]
    return out

